# revision 1
# baseline (speedup 1.0000x reference)
"""GQA attention (dense_transformer) on 8 TRN2 NeuronCores — v3.

Sharding: tensor-parallel over q heads. Core c owns q-heads {2c, 2c+1} and
their shared kv head c//2: column-parallel Wq/Wk/Wv, row-parallel Wo; the 8
partial o_proj outputs are summed on the host.

Per-core kernel strategy (all matmuls fp16 — same PE rate as bf16 but ~6x
less rounding error; fp8 DoubleRow was tried and rejected: for
random-sign contractions its ~4% quantization error does not average down,
and a single fp8 stage already exceeds the 2e-2 error budget):
  - projections: moving operand is a [128, 16, 512] fp16 x^T slab (one DMA
    per 512 tokens), stationary the k-tiled weights; f32 PSUM accumulate.
  - rotate-half (RoPE) is an SBUF->SBUF partition-swap DMA with the sign
    folded into the sin table; combine is 3 fp16 DVE ops per 512-chunk,
    emitted per projection block so phase A has no serial tail.
  - V^T -> V via DMA xbar transposes, spread across projection blocks.
  - scores (QK^T, contraction 128) in 2-PSUM-bank groups; exp on ACT at
    free-dim 1024, writing fp16 ex tiles.
  - softmax denominator: log2-tree folds of ex (GpSimd takes the two big
    folds, DVE the rest) + one all-ones matmul, instead of 16 all-ones
    matmuls per sq-group on the PE.
  - o_proj emitted after each (batch, sq-group)'s two head units so its
    matmuls/copies overlap the remaining attention work.
"""

import math

import numpy as np

import concourse.bacc as bacc_mod
import concourse.mybir as mybir
import concourse.tile as tile
from concourse.bass_utils import run_bass_kernel_spmd

HIDDEN = 2048
N_HEADS = 16
N_KV_HEADS = 4
HEAD_DIM = 128
ROPE_THETA = 10000.0
B = 2
S = 2048
N_CORES = 8
NH_LOC = N_HEADS // N_CORES  # 2 q heads per core
P = 128
KT = HIDDEN // P  # 16 contraction k-tiles over hidden
NSK = S // P  # 16 sk tiles per batch
F32 = mybir.dt.float32
FP16 = mybir.dt.float16
SCALE = 1.0 / math.sqrt(HEAD_DIM)


def _rope_tables(s, d):
    inv_freq = 1.0 / (ROPE_THETA ** (np.arange(0, d, 2, dtype=np.float32) / d))
    t = np.arange(s, dtype=np.float32)
    freqs = np.outer(t, inv_freq).astype(np.float32)  # [S, d/2]
    emb = np.concatenate([freqs, freqs], axis=-1)  # [S, d]
    cos_t = np.ascontiguousarray(np.cos(emb).T)  # [d, S]
    # rotate_half sign folded into sin rows: rows 0..63 multiply -x[64:128]
    sin_t = np.sin(emb).T.copy()
    sin_t[: d // 2, :] *= -1.0
    return cos_t.astype(np.float16), np.ascontiguousarray(sin_t).astype(np.float16)


def _ktile(a, p=P):
    """[K, M] -> [p, K//p, M] with contraction index = tile*p + partition."""
    k, m = a.shape
    return np.ascontiguousarray(a.reshape(k // p, p, m).transpose(1, 0, 2))


def _build(add_mask):
    nc = bacc_mod.Bacc()
    xt_d = nc.dram_tensor("xt", [P, KT, B * S], FP16, kind="ExternalInput")
    wq_d = nc.dram_tensor("wq", [P, KT, NH_LOC * P], FP16, kind="ExternalInput")
    wk_d = nc.dram_tensor("wk", [P, KT, P], FP16, kind="ExternalInput")
    wv_d = nc.dram_tensor("wv", [P, KT, P], FP16, kind="ExternalInput")
    wo_d = nc.dram_tensor("wo", [P, NH_LOC, HIDDEN], FP16, kind="ExternalInput")
    cos_d = nc.dram_tensor("cos_t", [P, S], FP16, kind="ExternalInput")
    sin_d = nc.dram_tensor("sin_t", [P, S], FP16, kind="ExternalInput")
    if add_mask:
        # mask transposed + k-tiled: [P, NSK, S] ([sk%P, sk//P, sq])
        mt_d = nc.dram_tensor("mask_t", [P, NSK, S], F32, kind="ExternalInput")
    out_d = nc.dram_tensor("out", [B * S, HIDDEN], FP16, kind="ExternalOutput")

    with tile.TileContext(nc) as tc:
        with (
            tc.tile_pool(name="consts", bufs=1) as consts,
            tc.tile_pool(name="persist", bufs=1) as persist,
        ):
            cos_sb = consts.tile([P, S], FP16, tag="cos")
            sin_sb = consts.tile([P, S], FP16, tag="sin")
            ones_sb = consts.tile([P, P], FP16, tag="ones")
            nc.gpsimd.memset(ones_sb, 1.0)
            wq_sb = consts.tile([P, KT, NH_LOC * P], FP16, tag="wq")
            wk_sb = consts.tile([P, KT, P], FP16, tag="wk")
            wv_sb = consts.tile([P, KT, P], FP16, tag="wv")
            wo_sb = consts.tile([P, NH_LOC, HIDDEN], FP16, tag="wo")
            nc.scalar.dma_start(out=wq_sb[:, 0:4, :], in_=wq_d[:, 0:4, :])
            nc.sync.dma_start(out=wq_sb[:, 4:KT, :], in_=wq_d[:, 4:KT, :])
            nc.sync.dma_start(out=wk_sb, in_=wk_d[:, :, :])
            nc.sync.dma_start(out=wv_sb, in_=wv_d[:, :, :])
            nc.gpsimd.dma_start(out=cos_sb, in_=cos_d[:, :])
            nc.gpsimd.dma_start(out=sin_sb, in_=sin_d[:, :])
            # warm the ACT exp table set during phase A
            scr = consts.tile([P, 16], FP16, tag="scr")
            nc.scalar.activation(
                scr, cos_sb[:, 0:16], mybir.ActivationFunctionType.Exp)

            qr = {}
            kr = {}
            vn = {}
            outn = {}
            for bi in range(B):
                for m in range(NH_LOC):
                    qr[(bi, m)] = persist.tile(
                        [P, S], FP16, tag=f"qr{bi}{m}", name=f"qr{bi}{m}")
                kr[bi] = persist.tile(
                    [P, S], FP16, tag=f"kr{bi}", name=f"kr{bi}")
                vn[bi] = persist.tile(
                    [P, NSK, P], FP16, tag=f"vn{bi}", name=f"vn{bi}")
                outn[bi] = persist.tile(
                    [P, NH_LOC, S], FP16, tag=f"on{bi}", name=f"on{bi}")

            # ---------------- Phase A: projections + RoPE + V^T ----------
            h = P // 2
            with (
                tc.tile_pool(name="stage_a", bufs=1) as st,
                tc.tile_pool(name="ps_a", bufs=1, space="PSUM") as pa,
            ):
                for bi in range(B):
                    q_st = [
                        st.tile([P, S], FP16, tag=f"qst{m}", bufs=2,
                                name=f"qst{m}")
                        for m in range(NH_LOC)
                    ]
                    k_st = st.tile([P, S], FP16, tag="kst", bufs=2, name="kst")
                    vt_st = st.tile([P, S], FP16, tag="vst", bufs=2, name="vst")
                    for blk in range(S // 512):
                        sl = slice(blk * 512, (blk + 1) * 512)
                        xt_sb = st.tile([P, KT, 512], FP16, tag="xt", bufs=3,
                                        name="xt_sb")
                        nc.gpsimd.dma_start(
                            out=xt_sb,
                            in_=xt_d[:, :, bi * S + blk * 512
                                     : bi * S + (blk + 1) * 512],
                        )
                        pp = pa.tile([P, NH_LOC + 2, 512], F32, tag="pp",
                                     bufs=2, name="pp")
                        for c in range(KT):
                            st_ = c == 0
                            sp_ = c == KT - 1
                            for m in range(NH_LOC):
                                nc.tensor.matmul(
                                    pp[:, m, :],
                                    wq_sb[:, c, m * P : (m + 1) * P],
                                    xt_sb[:, c, :],
                                    start=st_, stop=sp_,
                                )
                            nc.tensor.matmul(
                                pp[:, NH_LOC, :], wk_sb[:, c, :],
                                xt_sb[:, c, :], start=st_, stop=sp_,
                            )
                            nc.tensor.matmul(
                                pp[:, NH_LOC + 1, :], wv_sb[:, c, :],
                                xt_sb[:, c, :], start=st_, stop=sp_,
                            )
                        nc.scalar.copy(q_st[0][:, sl], pp[:, 0, :])
                        nc.scalar.copy(q_st[1][:, sl], pp[:, 1, :])
                        nc.scalar.copy(k_st[:, sl], pp[:, NH_LOC, :])
                        nc.scalar.copy(vt_st[:, sl], pp[:, NH_LOC + 1, :])

                        # RoPE for this block's q0/q1/k slices: rotate-half
                        # via partition-swap DMA (sign folded into sin).
                        for src, dst in [
                            (q_st[0], qr[(bi, 0)]),
                            (q_st[1], qr[(bi, 1)]),
                            (k_st, kr[bi]),
                        ]:
                            tq = st.tile([P, 512], FP16, tag="tq", bufs=8,
                                         name="tq")
                            nc.scalar.dma_start(out=tq[0:h, :],
                                                in_=src[h:P, sl])
                            nc.scalar.dma_start(out=tq[h:P, :],
                                                in_=src[0:h, sl])
                            tcs = st.tile([P, 512], FP16, tag="tcs", bufs=6,
                                          name="tcs")
                            nc.vector.tensor_mul(tcs, src[:, sl],
                                                 cos_sb[:, sl])
                            nc.vector.tensor_mul(tq, tq, sin_sb[:, sl])
                            nc.vector.tensor_add(dst[:, sl], tcs, tq)

                        # V^T -> V for this block via DMA xbar transpose
                        for j in range(4):
                            t = 4 * blk + j
                            nc.sync.dma_start_transpose(
                                vn[bi][:, t, :],
                                vt_st[:, t * P : (t + 1) * P])

            nc.scalar.dma_start(out=wo_sb, in_=wo_d[:, :, :])

            # ------------- Phases B (attention) + C (o_proj) --------------
            with (
                tc.tile_pool(name="stage_b", bufs=1) as sb,
                tc.tile_pool(name="ps_sc", bufs=1, space="PSUM") as psc,
                tc.tile_pool(name="ps_out", bufs=1, space="PSUM") as pout,
                tc.tile_pool(name="ps_sum", bufs=1, space="PSUM") as psum_,
                tc.tile_pool(name="ps_o", bufs=1, space="PSUM") as po_,
            ):
                for bi in range(B):
                    for sqg in range(S // 512):
                        qsl = slice(sqg * 512, (sqg + 1) * 512)
                        for m in range(NH_LOC):
                            ex = sb.tile([P, NSK, 512], FP16, tag="ex",
                                         bufs=2, name="ex")
                            out_ps = pout.tile([P, 512], F32, tag="out",
                                               bufs=1, name="out_ps")
                            sums_ps = psum_.tile([P, 512], F32, tag="sums",
                                                 bufs=1, name="sums_ps")
                            for g in range(NSK // 2):
                                g_sl = slice(2 * g, 2 * g + 2)
                                scg = psc.tile([P, 2, 512], F32, tag="sc",
                                               bufs=2, name="scg")
                                for j in range(2):
                                    t = 2 * g + j
                                    nc.tensor.matmul(
                                        scg[:, j, :],
                                        kr[bi][:, t * P : (t + 1) * P],
                                        qr[(bi, m)][:, qsl],
                                        start=True, stop=True,
                                    )
                                if add_mask:
                                    mk = sb.tile([P, 2, 512], F32, tag="mk",
                                                 bufs=4, name="mk")
                                    nc.sync.dma_start(
                                        out=mk, in_=mt_d[:, g_sl, qsl])
                                    nc.vector.scalar_tensor_tensor(
                                        scg, scg, SCALE, mk,
                                        op0=mybir.AluOpType.mult,
                                        op1=mybir.AluOpType.add,
                                    )
                                    nc.scalar.activation(
                                        ex[:, g_sl, :], scg,
                                        mybir.ActivationFunctionType.Exp,
                                    )
                                else:
                                    nc.scalar.activation(
                                        ex[:, g_sl, :], scg,
                                        mybir.ActivationFunctionType.Exp,
                                        scale=SCALE,
                                    )
                                for j in range(2):
                                    t = 2 * g + j
                                    nc.tensor.matmul(
                                        out_ps, vn[bi][:, t, :], ex[:, t, :],
                                        start=t == 0, stop=t == NSK - 1,
                                    )
                            # denominator: log2-tree fold of ex, then one
                            # all-ones matmul for the partition reduction
                            nc.vector.tensor_add(
                                ex[:, 0:8, :], ex[:, 0:8, :], ex[:, 8:16, :])
                            nc.vector.tensor_add(
                                ex[:, 0:4, :], ex[:, 0:4, :], ex[:, 4:8, :])
                            nc.vector.tensor_add(
                                ex[:, 0:2, :], ex[:, 0:2, :], ex[:, 2:4, :])
                            nc.vector.tensor_add(
                                ex[:, 0, :], ex[:, 0, :], ex[:, 1, :])
                            nc.tensor.matmul(
                                sums_ps, ones_sb, ex[:, 0, :],
                                start=True, stop=True,
                            )
                            onr = sb.tile([P, 512], F32, tag="onr", bufs=2,
                                          name="onr")
                            nc.scalar.copy(onr, out_ps)
                            rec = sb.tile([P, 512], F32, tag="rec", bufs=2,
                                          name="rec")
                            nc.vector.reciprocal_approx_fast(rec, sums_ps)
                            nc.vector.tensor_mul(
                                outn[bi][:, m, qsl], onr, rec)
                        # o_proj for the 4 sq-tiles this sq-group completed
                        for sqt in range(4 * sqg, 4 * sqg + 4):
                            po = po_.tile([P, 2, 512], F32, tag="po", bufs=1,
                                          name="po")
                            ob = sb.tile([P, HIDDEN], FP16, tag="ob", bufs=3,
                                         name="ob")
                            for half in range(2):
                                for hc in range(2):
                                    for dc in range(NH_LOC):
                                        nc.tensor.matmul(
                                            po[:, hc, :],
                                            outn[bi][:, dc,
                                                     sqt * P : (sqt + 1) * P],
                                            wo_sb[:, dc,
                                                  (2 * half + hc) * 512
                                                  : (2 * half + hc + 1) * 512],
                                            start=dc == 0,
                                            stop=dc == NH_LOC - 1,
                                        )
                                if bi == B - 1 and sqg == S // 512 - 1:
                                    nc.scalar.copy(
                                        ob[:, half * 1024 : (half + 1) * 1024],
                                        po)
                                else:
                                    nc.vector.tensor_copy(
                                        ob[:, half * 1024 : (half + 1) * 1024],
                                        po)
                            nc.sync.dma_start(
                                out=out_d[
                                    bi * S + sqt * P : bi * S + (sqt + 1) * P,
                                    :,
                                ],
                                in_=ob,
                            )
    nc.compile()
    return nc


_BUILD_CACHE = {}
LAST_RESULT = None


def _get_nc(add_mask):
    if add_mask not in _BUILD_CACHE:
        _BUILD_CACHE[add_mask] = _build(add_mask)
    return _BUILD_CACHE[add_mask]


def kernel(hidden_states, attention_mask, Wq, Wk, Wv, Wo):
    hidden_states = np.asarray(hidden_states, dtype=np.float32)
    attention_mask = np.asarray(attention_mask, dtype=np.float32)
    Wq = np.asarray(Wq, dtype=np.float32)
    Wk = np.asarray(Wk, dtype=np.float32)
    Wv = np.asarray(Wv, dtype=np.float32)
    Wo = np.asarray(Wo, dtype=np.float32)

    b, s, hidden = hidden_states.shape
    assert (b, s, hidden) == (B, S, HIDDEN)

    add_mask = bool(np.any(attention_mask))
    nc = _get_nc(add_mask)

    xt16 = _ktile(hidden_states.reshape(b * s, hidden).T.astype(np.float16))
    cos_t, sin_t = _rope_tables(s, HEAD_DIM)

    if add_mask:
        mt_kt = _ktile(np.ascontiguousarray(attention_mask[0, 0].T))

    in_maps = []
    for c in range(N_CORES):
        kv = c // 2
        im = {
            "xt": xt16,
            "cos_t": cos_t,
            "sin_t": sin_t,
            "wq": _ktile(
                Wq[:, c * NH_LOC * HEAD_DIM : (c + 1) * NH_LOC * HEAD_DIM]
                .astype(np.float16)),
            "wk": _ktile(Wk[:, kv * HEAD_DIM : (kv + 1) * HEAD_DIM]
                         .astype(np.float16)),
            "wv": _ktile(Wv[:, kv * HEAD_DIM : (kv + 1) * HEAD_DIM]
                         .astype(np.float16)),
            "wo": np.ascontiguousarray(
                Wo[c * NH_LOC * HEAD_DIM : (c + 1) * NH_LOC * HEAD_DIM, :]
                .astype(np.float16)
                .reshape(NH_LOC, P, HIDDEN).transpose(1, 0, 2)),
        }
        if add_mask:
            im["mask_t"] = mt_kt
        in_maps.append(im)

    res = run_bass_kernel_spmd(nc, in_maps, core_ids=list(range(N_CORES)))
    global LAST_RESULT
    LAST_RESULT = res
    out = np.zeros((b * s, hidden), dtype=np.float32)
    for r in res.results:
        out += np.asarray(r["out"], dtype=np.float32)
    return out.reshape(b, s, hidden)

